# revision 1
# baseline (speedup 1.0000x reference)
"""Causal self-attention Trainium2 kernel (8 NeuronCores, bf16 compute).

Sharding: core c -> batch b = c//4, head group hg = c%4 (4 heads each).
Each core computes its heads' QKV projections, causal attention, and a
partial output projection yt[d, t] (transposed, fp16). Host sums the 4
partials per batch, transposes, and adds b_proj.

Device dataflow per core:
  load    : x arrives span-major-packed (one DMA per 512-token span, all
            16 contraction stripes contiguous) so the first QKV group is
            gated on ~2MB, not the whole 8MB.
  per head: QT/KT/VT = W.T @ x chunks (transposed projections, hd on
            partitions); psum->sbuf copies (+bias) on the Vector engine
            so Scalar stays exclusive to exp.
            V = PE-transpose(VT)  (natural [tok, hd] layout)
            per q-span (512): for each k-block kj:
               ST[k,q] = KT_blk.T @ QT_span   (scores transposed, PSUM)
               += causal mask on diagonal blocks (DVE)
               PT = exp(scale*ST)             (ACT, bf16, unnormalized)
               sum[1,q]  += ones.T @ PT       (PE)
               OT[hd,q]  += V_blk.T @ PT      (PE)
            recipT = 1/sum (DVE), OT_sbuf = OT * recipT (DVE, bf16)
  proj    : yt[dc, t] += Wp_blk.T @ OT_h over heads -> fp16 -> DRAM
PSUM: acc pool (QKV groups + attn PV accumulator + proj) 3 banks,
score blocks 3 banks, softmax-sum accumulator 2 banks = 8 banks.
"""
import numpy as np

B, S, D, H = 2, 2048, 2048, 16
HD = 128
NCORES = 8
HPC = H // (NCORES // B)     # heads per core = 4
NEG = -1e9


def build_nc(S=S, D=D, nh=HPC, span=512):
    import concourse.bass as bass
    import concourse.mybir as mybir
    from concourse import bacc
    from concourse.tile import TileContext

    f32 = mybir.dt.float32
    f16 = mybir.dt.float16
    bf16 = mybir.dt.bfloat16
    KT = D // 128          # contraction tiles for qkv
    TT = S // 128          # token tiles
    NS = S // span         # q spans
    KPS = span // 128      # k-blocks per span
    SPW = KT * span        # packed x columns per span
    scale = float(HD) ** -0.5

    nc = bacc.Bacc("TRN2", target_bir_lowering=False, debug=False)
    x_d = nc.dram_tensor("xt", [128, NS * SPW], bf16, kind="ExternalInput").ap()
    wq_d = nc.dram_tensor("wqkv", [3 * nh * 128, D], bf16, kind="ExternalInput").ap()
    bq_d = nc.dram_tensor("bqkv", [128, 3 * nh], f32, kind="ExternalInput").ap()
    wp_d = nc.dram_tensor("wproj", [nh * 128, D], bf16, kind="ExternalInput").ap()
    tm_d = nc.dram_tensor("trimaskT", [128, 128], f32, kind="ExternalInput").ap()
    id_d = nc.dram_tensor("identb", [128, 128], bf16, kind="ExternalInput").ap()
    oc_d = nc.dram_tensor("ones_sq", [128, 128], bf16, kind="ExternalInput").ap()
    yt_d = nc.dram_tensor("yt", [D, S], f16, kind="ExternalOutput").ap()

    Act = mybir.ActivationFunctionType
    Alu = mybir.AluOpType

    with TileContext(nc) as tc:
        from contextlib import ExitStack
        with ExitStack() as ctx:
            res = ctx.enter_context(tc.tile_pool(name="res", bufs=1))
            w_p = ctx.enter_context(tc.tile_pool(name="w", bufs=1))
            wp_p = ctx.enter_context(tc.tile_pool(name="wp", bufs=1))
            qk_p = ctx.enter_context(tc.tile_pool(name="qk", bufs=2))
            v_p = ctx.enter_context(tc.tile_pool(name="v", bufs=2))
            pt_p = ctx.enter_context(tc.tile_pool(name="pt", bufs=4))
            sm_p = ctx.enter_context(tc.tile_pool(name="sm", bufs=4))
            yst_p = ctx.enter_context(tc.tile_pool(name="yst", bufs=2))
            ps_t = ctx.enter_context(tc.tile_pool(name="ps_t", bufs=2, space="PSUM"))
            ps_acc = ctx.enter_context(tc.tile_pool(name="ps_acc", bufs=2, space="PSUM"))
            ps_st = ctx.enter_context(tc.tile_pool(name="ps_st", bufs=3, space="PSUM"))
            ps_sm = ctx.enter_context(tc.tile_pool(name="ps_sm", bufs=1, space="PSUM"))

            # ---- startup-critical DMA order: head-0 weights + x span 0
            # first, constants after.  One DMA per x span (contiguous). ----
            wstrip = {}

            def w_dma(h, p, name):
                wt = w_p.tile([128, D], bf16, tag=f"w{h}_{p}", name=name)
                nc.sync.dma_start(
                    wt, wq_d[(p * nh + h) * 128:(p * nh + h + 1) * 128, :])
                wstrip[(h, p)] = wt

            xall = res.tile([128, NS * SPW], bf16, tag="xall")
            w_dma(0, 0, "w0_0")
            # span 0 in two chunks so early kt stripes land sooner
            nc.sync.dma_start(xall[:, :SPW // 2], x_d[:, :SPW // 2])
            nc.sync.dma_start(xall[:, SPW // 2:SPW], x_d[:, SPW // 2:SPW])
            w_dma(0, 1, "w0_1")
            w_dma(0, 2, "w0_2")
            bq = res.tile([128, 3 * nh], f32, tag="bq")
            nc.sync.dma_start(bq, bq_d)
            if NS > 1:
                nc.sync.dma_start(xall[:, SPW:2 * SPW], x_d[:, SPW:2 * SPW])
            trimaskT = res.tile([128, 128], f32, tag="trimaskT")
            identb = res.tile([128, 128], bf16, tag="identb")
            ones_sq = res.tile([128, 128], bf16, tag="ones_sq")
            nc.sync.dma_start(trimaskT, tm_d)
            nc.sync.dma_start(identb, id_d)
            nc.sync.dma_start(ones_sq, oc_d)
            for sp in range(2, NS):
                nc.sync.dma_start(
                    xall[:, sp * SPW:(sp + 1) * SPW],
                    x_d[:, sp * SPW:(sp + 1) * SPW])
            # remaining weights: dedicated tiles, all loaded up front so no
            # input DMA lands mid-run (concurrent DMA writes throttle PE)
            for h in range(1, nh):
                for p in range(3):
                    w_dma(h, p, f"w{h}_{p}")
            wp = []
            for g in range(nh):
                w = wp_p.tile([128, D], bf16, tag=f"wpt{g}")
                nc.sync.dma_start(w, wp_d[g * 128:(g + 1) * 128, :])
                wp.append(w)

            # ---- per-head OT accumulation ----
            OT = [res.tile([128, S], bf16, tag=f"ot{h}", name=f"ot{h}")
                  for h in range(nh)]

            def qkv_group(wt, dst, hp, sp):
                ps = ps_acc.tile([128, span], f32, tag="acc")
                for kt in range(KT):
                    nc.tensor.matmul(
                        ps, wt[:, kt * 128:(kt + 1) * 128],
                        xall[:, sp * SPW + kt * span:
                             sp * SPW + (kt + 1) * span],
                        start=(kt == 0), stop=(kt == KT - 1))
                nc.vector.tensor_scalar(
                    out=dst[:, sp * span:(sp + 1) * span], in0=ps,
                    scalar1=bq[:, hp:hp + 1], scalar2=None, op0=Alu.add)

            preQT = {}
            for h in range(nh):
                wts = [wstrip.pop((h, p)) for p in range(3)]
                # qkv projections (transposed: [hd, tok]); span-major so the
                # startup x-span DMAs hide behind 3 projections per span
                qt = preQT.pop(h, None)
                pre = qt is not None
                if qt is None:
                    qt = qk_p.tile([128, S], bf16, tag="qt", name=f"qt{h}")
                kt_ = qk_p.tile([128, S], bf16, tag="kt_", name=f"kt_{h}")
                vt = qk_p.tile([128, S], bf16, tag="vt", name=f"vt{h}")
                qkvT = [qt, kt_, vt]
                for sp in range(NS):
                    for p in range(3):
                        if pre and sp == 0 and p == 0:
                            continue   # Q span 0 already emitted last head
                        qkv_group(wts[p], qkvT[p], p * nh + h, sp)
                QT, KTt, VT = qkvT

                # V natural [tok, hd]: PE-transpose VT in groups of 4
                vh = v_p.tile([128, S], bf16, tag="v")
                for tg in range(0, TT, 4):
                    n = min(4, TT - tg)
                    ps = ps_t.tile([128, 512], bf16, tag="tp")
                    for j in range(n):
                        nc.tensor.transpose(
                            ps[:, j * 128:(j + 1) * 128],
                            VT[:, (tg + j) * 128:(tg + j + 1) * 128], identb)
                    nc.vector.tensor_copy(
                        vh[:, tg * 128:(tg + n) * 128], ps[:, :n * 128])

                # attention per q-span
                for sp in range(NS):
                    nkj = KPS * (sp + 1)   # causal: k-blocks 0..nkj-1
                    ps_o = ps_acc.tile([128, span], f32, tag="acc")
                    ps_s = ps_sm.tile([128, span], f32, tag="sum")
                    pend = []  # (kj, pt, qoff) awaiting sum/av emission

                    def flush_one():
                        kj, pt, qoff = pend.pop(0)
                        nc.tensor.matmul(
                            ps_s[:, qoff:], ones_sq, pt[:, qoff:],
                            start=(kj == 0), stop=(kj == nkj - 1))
                        nc.tensor.matmul(
                            ps_o[:, qoff:], vh[:, kj * 128:(kj + 1) * 128],
                            pt[:, qoff:], start=(kj == 0), stop=(kj == nkj - 1))

                    for kj in range(nkj):
                        qoff = max(0, (kj - KPS * sp)) * 128
                        ps = ps_st.tile([128, span], f32, tag="st")
                        nc.tensor.matmul(
                            ps[:, qoff:], KTt[:, kj * 128:(kj + 1) * 128],
                            QT[:, sp * span + qoff:(sp + 1) * span],
                            start=True, stop=True)
                        if kj >= KPS * sp:  # diagonal block: causal mask
                            nc.vector.tensor_tensor(
                                out=ps[:, qoff:qoff + 128],
                                in0=ps[:, qoff:qoff + 128],
                                in1=trimaskT, op=Alu.add)
                        pt = pt_p.tile([128, span], bf16, tag="pt")
                        nc.scalar.activation(
                            pt[:, qoff:], ps[:, qoff:], Act.Exp, scale=scale)
                        pend.append((kj, pt, qoff))
                        if len(pend) > 2:
                            flush_one()
                    while pend:
                        flush_one()

                    recipb = sm_p.tile([128, span], f32, tag="recipb")
                    nc.vector.reciprocal_approx_fast(out=recipb, in_=ps_s)
                    nc.vector.tensor_tensor(
                        out=OT[h][:, sp * span:(sp + 1) * span],
                        in0=ps_o, in1=recipb, op=Alu.mult)

            # ---- output projection: yt[dc, t] = sum_h Wp_h.T @ OT_h ----
            for dc in range(D // 128):
                yst = yst_p.tile([128, S], f16, tag="yst")
                for sp in range(NS):
                    ps = ps_acc.tile([128, span], f32, tag="acc")
                    for h in range(nh):
                        nc.tensor.matmul(
                            ps, wp[h][:, dc * 128:(dc + 1) * 128],
                            OT[h][:, sp * span:(sp + 1) * span],
                            start=(h == 0), stop=(h == nh - 1))
                    nc.vector.tensor_copy(yst[:, sp * span:(sp + 1) * span], ps)
                    # chunked DMA-out: drain starts before the stripe finishes
                    nc.sync.dma_start(
                        yt_d[dc * 128:(dc + 1) * 128,
                             sp * span:(sp + 1) * span],
                        yst[:, sp * span:(sp + 1) * span])

    nc.finalize()
    return nc


def pack_x(xb, S=S, D=D, span=512):
    """[S, D] -> span-major packed [128, NS*KT*span] (bf16 upstream)."""
    NS, KT = S // span, D // 128
    return np.ascontiguousarray(
        xb.reshape(NS, span, KT, 128).transpose(3, 0, 2, 1).reshape(128, -1))


def _prep_core_inputs(x, W_qkv, b_qkv, W_proj, core, S=S, D=D, nh=HPC):
    import ml_dtypes
    bf16 = ml_dtypes.bfloat16
    ngr = NCORES // B
    b, hg = core // ngr, core % ngr
    KT = D // 128
    Dfull = W_qkv.shape[0]

    wq = np.empty((3 * nh * 128, D), dtype=bf16)
    bq = np.zeros((128, 3 * nh), dtype=np.float32)
    for p in range(3):
        for h in range(nh):
            g = hg * nh + h
            col = p * Dfull + g * 128
            blk = W_qkv[:, col:col + 128]            # [D, 128]
            hp = p * nh + h
            wq[hp * 128:(hp + 1) * 128] = (
                blk.reshape(KT, 128, 128).transpose(1, 0, 2).reshape(128, D)
                .astype(bf16))
            bq[:, hp] = b_qkv[col:col + 128]
    wp = W_proj[hg * nh * 128:(hg + 1) * nh * 128, :].astype(bf16)

    r = np.arange(128)
    trimaskT = np.where(r[:, None] <= r[None, :], 0.0, NEG).astype(np.float32)
    return {
        "xt": pack_x(x[b]).astype(bf16),
        "wqkv": wq,
        "bqkv": bq,
        "wproj": wp,
        "trimaskT": trimaskT,
        "identb": np.eye(128, dtype=bf16),
        "ones_sq": np.ones((128, 128), dtype=bf16),
    }


_CACHE = {}


def kernel(x, W_qkv, b_qkv, W_proj, b_proj, mask):
    from concourse.bass_utils import run_bass_kernel_spmd

    x = np.asarray(x)
    W_qkv = np.asarray(W_qkv)
    b_qkv = np.asarray(b_qkv)
    W_proj = np.asarray(W_proj)
    b_proj = np.asarray(b_proj)

    if "nc" not in _CACHE:
        _CACHE["nc"] = build_nc()
    nc = _CACHE["nc"]

    in_maps = [_prep_core_inputs(x, W_qkv, b_qkv, W_proj, c)
               for c in range(NCORES)]
    res = run_bass_kernel_spmd(nc, in_maps, core_ids=list(range(NCORES)))

    ngr = NCORES // B
    out = np.empty((B, S, D), dtype=np.float32)
    for b in range(B):
        acc = res.results[b * ngr]["yt"].astype(np.float32)
        for g in range(1, ngr):
            acc = acc + res.results[b * ngr + g]["yt"]
        out[b] = acc.T + b_proj[None, :]
    return out



# revision 4
# speedup vs baseline: 1.0139x; 1.0139x over previous
"""Causal self-attention Trainium2 kernel (8 NeuronCores, bf16 compute).

Sharding: core c -> batch b = c//4, head group hg = c%4 (4 heads each).
Each core computes its heads' QKV projections, causal attention, and a
partial output projection yt[d, t] (transposed, fp16). Host sums the 4
partials per batch, transposes, and adds b_proj.

Device dataflow per core (v2):
  startup : ~24 zero matmuls warm the PE clock (HAM) while the first
            x/weight DMAs land, split across 4 DMA queues (sync/act/
            dve/pool issue) so span-0 x + head-0 weights arrive ~2.5us.
  per head: QT/KT/VT = W.T @ x chunks (transposed projections, hd on
            partitions); psum->sbuf copies (+bias) on the Vector engine.
            V = PE-transpose(VT) in two batches of <=8 blocks (one PSUM
            bank), batch 2 deferred to mid-attention.
            per q-span (512): for each k-block kj:
               ST[k,q] = KT_blk.T @ QT_span   (PSUM, f32)
               PT = exp(scale*ST)             (ACT, bf16, unnormalized)
               diag blocks: PT *= tri         (GpSimd, multiplicative)
               acc += PT                      (DVE, fp16 accumulator)
               OT[hd,q] += V_blk.T @ PT       (PE)
            span end: sums = ones16.T @ acc (PE, broadcast over hd),
            recipT = 1/sums (DVE), OT_sbuf = OT * recipT (DVE, bf16)
  pipeline: next head's QKV matmuls are queued as thunks and injected
            one per attention block, so the exp-gated attention phase
            keeps the Tensor engine busy; the remainder drains between
            heads. During the last head, early proj stripes inject.
  proj    : yt[dc, t] += Wp_blk.T @ OT_h over heads -> fp16 -> DRAM
PSUM banks: qkv 2, pv 1, scores 3, transpose 1, sums 1 = 8.
"""
import numpy as np
from collections import deque

B, S, D, H = 2, 2048, 2048, 16
HD = 128
NCORES = 8
HPC = H // (NCORES // B)     # heads per core = 4
WARMUP = 24


def build_nc(S=S, D=D, nh=HPC, span=512):
    import concourse.bass as bass
    import concourse.mybir as mybir
    from concourse import bacc
    from concourse.tile import TileContext

    f32 = mybir.dt.float32
    f16 = mybir.dt.float16
    bf16 = mybir.dt.bfloat16
    KT = D // 128          # contraction tiles for qkv
    TT = S // 128          # token tiles
    NS = S // span         # q spans
    KPS = span // 128      # k-blocks per span
    SPW = KT * span        # packed x columns per span
    scale = float(HD) ** -0.5

    nc = bacc.Bacc("TRN2", target_bir_lowering=False, debug=False)
    x_d = nc.dram_tensor("xt", [128, NS * SPW], bf16, kind="ExternalInput").ap()
    wq_d = nc.dram_tensor("wqkv", [3 * nh * 128, D], bf16, kind="ExternalInput").ap()
    bq_d = nc.dram_tensor("bqkv", [128, 3 * nh], f32, kind="ExternalInput").ap()
    wp_d = nc.dram_tensor("wproj", [nh * 128, D], bf16, kind="ExternalInput").ap()
    tb_d = nc.dram_tensor("tribin", [128, 128], bf16, kind="ExternalInput").ap()
    id_d = nc.dram_tensor("identb", [128, 128], bf16, kind="ExternalInput").ap()
    on_d = nc.dram_tensor("ones16", [128, 128], f16, kind="ExternalInput").ap()
    yt_d = nc.dram_tensor("yt", [D, S], f16, kind="ExternalOutput").ap()

    Act = mybir.ActivationFunctionType
    Alu = mybir.AluOpType

    with TileContext(nc) as tc:
        from contextlib import ExitStack
        with ExitStack() as ctx:
            res = ctx.enter_context(tc.tile_pool(name="res", bufs=1))
            w_p = ctx.enter_context(tc.tile_pool(name="w", bufs=1))
            wp_p = ctx.enter_context(tc.tile_pool(name="wp", bufs=1))
            qk_p = ctx.enter_context(tc.tile_pool(name="qk", bufs=2))
            v_p = ctx.enter_context(tc.tile_pool(name="v", bufs=2))
            pt_p = ctx.enter_context(tc.tile_pool(name="pt", bufs=6))
            acc_p = ctx.enter_context(tc.tile_pool(name="acc", bufs=2))
            sm_p = ctx.enter_context(tc.tile_pool(name="sm", bufs=2))
            yst_p = ctx.enter_context(tc.tile_pool(name="yst", bufs=3))
            ps_qkv = ctx.enter_context(tc.tile_pool(name="ps_qkv", bufs=2, space="PSUM"))
            ps_pv = ctx.enter_context(tc.tile_pool(name="ps_pv", bufs=1, space="PSUM"))
            ps_st = ctx.enter_context(tc.tile_pool(name="ps_st", bufs=3, space="PSUM"))
            ps_t = ctx.enter_context(tc.tile_pool(name="ps_t", bufs=1, space="PSUM"))
            ps_sm = ctx.enter_context(tc.tile_pool(name="ps_sm", bufs=1, space="PSUM"))

            # ---- PE warmup: zero matmuls (no DMA deps) ramp the HAM
            # clock while the first input DMAs are in flight. ----
            warm = res.tile([128, 128], bf16, tag="warm")
            nc.vector.memset(warm, 0.0)
            wps = ps_st.tile([128, span], f32, tag="st")
            for _ in range(WARMUP):
                nc.tensor.matmul(wps[:, :128], warm, warm, start=True, stop=True)

            # ---- startup DMAs split over 4 queues (issuing engine = queue):
            #   sync/vector: x (span 0 in kt chunks, then span halves)
            #   scalar     : qkv weights    gpsimd: bias, consts, wproj ----
            wstrip = {}

            def w_dma(h, p):
                wt = w_p.tile([128, D], bf16, tag=f"w{h}_{p}", name=f"w{h}_{p}")
                nc.scalar.dma_start(
                    wt, wq_d[(p * nh + h) * 128:(p * nh + h + 1) * 128, :])
                wstrip[(h, p)] = wt

            xall = res.tile([128, NS * SPW], bf16, tag="xall")
            bq = res.tile([128, 3 * nh], f32, tag="bq")
            tribin = res.tile([128, 128], bf16, tag="tribin")
            identb = res.tile([128, 128], bf16, tag="identb")
            ones16 = res.tile([128, 128], f16, tag="ones16")
            nc.gpsimd.dma_start(bq, bq_d)
            nchunk = min(4, KT)
            ktper = KT // nchunk
            xeng = [nc.sync, nc.gpsimd]
            for c in range(nchunk):
                xeng[c % 2].dma_start(
                    xall[:, c * ktper * span:(c + 1) * ktper * span],
                    x_d[:, c * ktper * span:(c + 1) * ktper * span])
            w_dma(0, 0)
            w_dma(0, 1)
            w_dma(0, 2)
            nc.gpsimd.dma_start(tribin, tb_d)
            nc.gpsimd.dma_start(identb, id_d)
            nc.gpsimd.dma_start(ones16, on_d)
            for sp in range(1, NS):
                half = SPW // 2
                nc.sync.dma_start(
                    xall[:, sp * SPW:sp * SPW + half],
                    x_d[:, sp * SPW:sp * SPW + half])
                nc.gpsimd.dma_start(
                    xall[:, sp * SPW + half:(sp + 1) * SPW],
                    x_d[:, sp * SPW + half:(sp + 1) * SPW])
            for h in range(1, nh):
                for p in range(3):
                    w_dma(h, p)
            wp = []
            for g in range(nh):
                w = wp_p.tile([128, D], bf16, tag=f"wpt{g}")
                nc.scalar.dma_start(w, wp_d[g * 128:(g + 1) * 128, :])
                wp.append(w)

            # ---- per-head OT accumulation ----
            OT = [res.tile([128, S], bf16, tag=f"ot{h}", name=f"ot{h}")
                  for h in range(nh)]

            # ---- deferred-PE work queue: QKV matmuls of the next head
            # (and early proj stripes during the last head) are emitted
            # one per attention block to fill exp-gated PE idle time. ----
            work_q = deque()

            def pull(n):
                for _ in range(min(n, len(work_q))):
                    work_q.popleft()()

            def drain_q():
                while work_q:
                    work_q.popleft()()

            def enqueue_qkv(h):
                wts = [wstrip.pop((h, p)) for p in range(3)]
                qt = qk_p.tile([128, S], bf16, tag="qt", name=f"qt{h}")
                kt_ = qk_p.tile([128, S], bf16, tag="kt_", name=f"kt_{h}")
                vt = qk_p.tile([128, S], bf16, tag="vt", name=f"vt{h}")
                qkvT = [qt, kt_, vt]
                for sp in range(NS):
                    for p in range(3):
                        cell = {}
                        for kt in range(KT):
                            def mm(kt=kt, sp=sp, wt=wts[p], cell=cell):
                                if kt == 0:
                                    cell['ps'] = ps_qkv.tile(
                                        [128, span], f32, tag="qkv",
                                        name=f"qg{sp}")
                                nc.tensor.matmul(
                                    cell['ps'], wt[:, kt * 128:(kt + 1) * 128],
                                    xall[:, sp * SPW + kt * span:
                                         sp * SPW + (kt + 1) * span],
                                    start=(kt == 0), stop=(kt == KT - 1))
                            work_q.append(mm)

                        def ev(sp=sp, hp=p * nh + h, cell=cell, dst=qkvT[p]):
                            nc.vector.tensor_scalar(
                                out=dst[:, sp * span:(sp + 1) * span],
                                in0=cell['ps'], scalar1=bq[:, hp:hp + 1],
                                scalar2=None, op0=Alu.add)
                        work_q.append(ev)
                return qkvT

            # proj: one stripe = 4 head-matmuls + cast + dma for (dc, sp)
            proj_done = set()

            def proj_stripe(dc, sp, thunks):
                cell = {}
                for g in range(nh):
                    def mm(g=g, dc=dc, sp=sp, cell=cell):
                        if g == 0:
                            cell['ps'] = ps_qkv.tile([128, span], f32,
                                                     tag="qkv", name=f"pj{dc}_{sp}")
                        nc.tensor.matmul(
                            cell['ps'], wp[g][:, dc * 128:(dc + 1) * 128],
                            OT[g][:, sp * span:(sp + 1) * span],
                            start=(g == 0), stop=(g == nh - 1))
                    thunks.append(mm)

                def ev(dc=dc, sp=sp, cell=cell):
                    yst = yst_p.tile([128, span], f16, tag="yst",
                                     name=f"yst{dc}_{sp}")
                    nc.vector.tensor_copy(yst, cell['ps'])
                    nc.sync.dma_start(
                        yt_d[dc * 128:(dc + 1) * 128,
                             sp * span:(sp + 1) * span], yst)
                thunks.append(ev)
                proj_done.add((dc, sp))

            # ---- head loop ----
            qkvT_next = enqueue_qkv(0)
            drain_q()
            for h in range(nh):
                QT, KTt, VT = qkvT_next
                if h + 1 < nh:
                    qkvT_next = enqueue_qkv(h + 1)

                vh = v_p.tile([128, S], bf16, tag="v")

                def transpose_batch(tg0, tgn):
                    ps = ps_t.tile([128, 1024], bf16, tag="tp")
                    for j in range(tgn - tg0):
                        nc.tensor.transpose(
                            ps[:, j * 128:(j + 1) * 128],
                            VT[:, (tg0 + j) * 128:(tg0 + j + 1) * 128], identb)
                    nc.vector.tensor_copy(
                        vh[:, tg0 * 128:tgn * 128], ps[:, :(tgn - tg0) * 128])

                transpose_batch(0, min(8, TT))

                for sp in range(NS):
                    if sp == 2 and TT > 8:
                        transpose_batch(8, TT)
                    nkj = KPS * (sp + 1)   # causal: k-blocks 0..nkj-1
                    ps_o = ps_pv.tile([128, span], f32, tag="pv")
                    acc = acc_p.tile([128, span], f16, tag="acc")
                    pend = []  # (kj, pt, qoff) awaiting PV emission

                    def flush_one(nkj=None, ps_o=None):
                        kj, pt, qoff = pend.pop(0)
                        nc.tensor.matmul(
                            ps_o[:, qoff:], vh[:, kj * 128:(kj + 1) * 128],
                            pt[:, qoff:], start=(kj == 0), stop=(kj == nkj - 1))

                    for kj in range(nkj):
                        qoff = max(0, (kj - KPS * sp)) * 128
                        st = ps_st.tile([128, span], f32, tag="st")
                        nc.tensor.matmul(
                            st[:, qoff:], KTt[:, kj * 128:(kj + 1) * 128],
                            QT[:, sp * span + qoff:(sp + 1) * span],
                            start=True, stop=True)
                        pull(1)
                        pt = pt_p.tile([128, span], bf16, tag="pt")
                        nc.scalar.activation(
                            pt[:, qoff:], st[:, qoff:], Act.Exp, scale=scale)
                        if kj >= KPS * sp:  # diagonal block: causal mask
                            nc.gpsimd.tensor_tensor(
                                out=pt[:, qoff:qoff + 128],
                                in0=pt[:, qoff:qoff + 128],
                                in1=tribin, op=Alu.mult)
                        if kj == 0:
                            nc.vector.tensor_copy(acc, pt)
                        else:
                            nc.vector.tensor_tensor(
                                out=acc[:, qoff:], in0=acc[:, qoff:],
                                in1=pt[:, qoff:], op=Alu.add)
                        pend.append((kj, pt, qoff))
                        if len(pend) > 3:
                            flush_one(nkj=nkj, ps_o=ps_o)
                    while pend:
                        flush_one(nkj=nkj, ps_o=ps_o)
                    pull(2)

                    ps_s = ps_sm.tile([128, span], f32, tag="sum")
                    nc.tensor.matmul(ps_s, ones16, acc, start=True, stop=True)
                    recipb = sm_p.tile([128, span], f32, tag="recipb")
                    nc.vector.reciprocal_approx_fast(out=recipb, in_=ps_s)
                    nc.vector.tensor_tensor(
                        out=OT[h][:, sp * span:(sp + 1) * span],
                        in0=ps_o, in1=recipb, op=Alu.mult)

                    if h == nh - 1:
                        # inject early proj stripes (bounded by DVE slack)
                        if sp == 0:
                            for dc in range(min(8, D // 128)):
                                proj_stripe(dc, 0, work_q)
                        elif sp == 1:
                            for dc in range(8, min(12, D // 128)):
                                proj_stripe(dc, 0, work_q)

                drain_q()

            # ---- remaining output projection ----
            for dc in range(D // 128):
                for sp in range(NS):
                    if (dc, sp) in proj_done:
                        continue
                    thunks = []
                    proj_stripe(dc, sp, thunks)
                    for t in thunks:
                        t()

    nc.finalize()
    return nc


def pack_x(xb, S=S, D=D, span=512):
    """[S, D] -> span-major packed [128, NS*KT*span] (bf16 upstream)."""
    NS, KT = S // span, D // 128
    return np.ascontiguousarray(
        xb.reshape(NS, span, KT, 128).transpose(3, 0, 2, 1).reshape(128, -1))


def _prep_core_inputs(x, W_qkv, b_qkv, W_proj, core, S=S, D=D, nh=HPC):
    import ml_dtypes
    bf16 = ml_dtypes.bfloat16
    ngr = NCORES // B
    b, hg = core // ngr, core % ngr
    KT = D // 128
    Dfull = W_qkv.shape[0]

    wq = np.empty((3 * nh * 128, D), dtype=bf16)
    bq = np.zeros((128, 3 * nh), dtype=np.float32)
    for p in range(3):
        for h in range(nh):
            g = hg * nh + h
            col = p * Dfull + g * 128
            blk = W_qkv[:, col:col + 128]            # [D, 128]
            hp = p * nh + h
            wq[hp * 128:(hp + 1) * 128] = (
                blk.reshape(KT, 128, 128).transpose(1, 0, 2).reshape(128, D)
                .astype(bf16))
            bq[:, hp] = b_qkv[col:col + 128]
    wp = W_proj[hg * nh * 128:(hg + 1) * nh * 128, :].astype(bf16)

    r = np.arange(128)
    tribin = np.where(r[:, None] <= r[None, :], 1.0, 0.0)
    return {
        "xt": pack_x(x[b]).astype(bf16),
        "wqkv": wq,
        "bqkv": bq,
        "wproj": wp,
        "tribin": tribin.astype(bf16),
        "identb": np.eye(128, dtype=bf16),
        "ones16": np.ones((128, 128), dtype=np.float16),
    }


_CACHE = {}


def kernel(x, W_qkv, b_qkv, W_proj, b_proj, mask):
    from concourse.bass_utils import run_bass_kernel_spmd

    x = np.asarray(x)
    W_qkv = np.asarray(W_qkv)
    b_qkv = np.asarray(b_qkv)
    W_proj = np.asarray(W_proj)
    b_proj = np.asarray(b_proj)

    if "nc" not in _CACHE:
        _CACHE["nc"] = build_nc()
    nc = _CACHE["nc"]

    in_maps = [_prep_core_inputs(x, W_qkv, b_qkv, W_proj, c)
               for c in range(NCORES)]
    res = run_bass_kernel_spmd(nc, in_maps, core_ids=list(range(NCORES)))

    ngr = NCORES // B
    out = np.empty((B, S, D), dtype=np.float32)
    for b in range(B):
        acc = res.results[b * ngr]["yt"].astype(np.float32)
        for g in range(1, ngr):
            acc = acc + res.results[b * ngr + g]["yt"]
        out[b] = acc.T + b_proj[None, :]
    return out


# revision 6
# speedup vs baseline: 1.0203x; 1.0063x over previous
"""Causal self-attention Trainium2 kernel (8 NeuronCores, bf16 compute).

Sharding: core c -> batch b = c//4, head group hg = c%4 (4 heads each).
Each core computes its heads' QKV projections, causal attention, and a
partial output projection yt[d, t] (transposed, fp16). Host sums the 4
partials per batch, transposes, and adds b_proj.

Device dataflow per core (v2):
  startup : ~24 zero matmuls warm the PE clock (HAM) while the first
            x/weight DMAs land, split across 4 DMA queues (sync/act/
            dve/pool issue) so span-0 x + head-0 weights arrive ~2.5us.
  per head: QT/KT/VT = W.T @ x chunks (transposed projections, hd on
            partitions); psum->sbuf copies (+bias) on the Vector engine.
            V = PE-transpose(VT) in two batches of <=8 blocks (one PSUM
            bank), batch 2 deferred to mid-attention.
            per q-span (512): for each k-block kj:
               ST[k,q] = KT_blk.T @ QT_span   (PSUM, f32)
               PT = exp(scale*ST)             (ACT, bf16, unnormalized)
               diag blocks: PT *= tri         (GpSimd, multiplicative)
               acc += PT                      (DVE, fp16 accumulator)
               OT[hd,q] += V_blk.T @ PT       (PE)
            span end: sums = ones16.T @ acc (PE, broadcast over hd),
            recipT = 1/sums (DVE), OT_sbuf = OT * recipT (DVE, bf16)
  pipeline: next head's QKV matmuls are queued as thunks and injected
            one per attention block, so the exp-gated attention phase
            keeps the Tensor engine busy; the remainder drains between
            heads. During the last head, early proj stripes inject.
  proj    : yt[dc, t] += Wp_blk.T @ OT_h over heads -> fp16 -> DRAM
PSUM banks: qkv 2, pv 1, scores 3, transpose 1, sums 1 = 8.
"""
import numpy as np
from collections import deque

B, S, D, H = 2, 2048, 2048, 16
HD = 128
NCORES = 8
HPC = H // (NCORES // B)     # heads per core = 4
WARMUP = 24


def build_nc(S=S, D=D, nh=HPC, span=512):
    import concourse.bass as bass
    import concourse.mybir as mybir
    from concourse import bacc
    from concourse.tile import TileContext

    f32 = mybir.dt.float32
    f16 = mybir.dt.float16
    bf16 = mybir.dt.bfloat16
    KT = D // 128          # contraction tiles for qkv
    TT = S // 128          # token tiles
    NS = S // span         # q spans
    KPS = span // 128      # k-blocks per span
    SPW = KT * span        # packed x columns per span
    scale = float(HD) ** -0.5

    nc = bacc.Bacc("TRN2", target_bir_lowering=False, debug=False)
    x_d = nc.dram_tensor("xt", [128, NS * SPW], bf16, kind="ExternalInput").ap()
    wq_d = nc.dram_tensor("wqkv", [3 * nh * 128, D], bf16, kind="ExternalInput").ap()
    bq_d = nc.dram_tensor("bqkv", [128, 3 * nh], f32, kind="ExternalInput").ap()
    wp_d = nc.dram_tensor("wproj", [nh * 128, D], bf16, kind="ExternalInput").ap()
    tb_d = nc.dram_tensor("tribin", [128, 128], bf16, kind="ExternalInput").ap()
    id_d = nc.dram_tensor("identb", [128, 128], bf16, kind="ExternalInput").ap()
    on_d = nc.dram_tensor("ones16", [128, 128], f16, kind="ExternalInput").ap()
    yt_d = nc.dram_tensor("yt", [D, S], f16, kind="ExternalOutput").ap()

    Act = mybir.ActivationFunctionType
    Alu = mybir.AluOpType

    with TileContext(nc) as tc:
        from contextlib import ExitStack
        with ExitStack() as ctx:
            res = ctx.enter_context(tc.tile_pool(name="res", bufs=1))
            w_p = ctx.enter_context(tc.tile_pool(name="w", bufs=1))
            wp_p = ctx.enter_context(tc.tile_pool(name="wp", bufs=1))
            qk_p = ctx.enter_context(tc.tile_pool(name="qk", bufs=2))
            v_p = ctx.enter_context(tc.tile_pool(name="v", bufs=2))
            pt_p = ctx.enter_context(tc.tile_pool(name="pt", bufs=7))
            acc_p = ctx.enter_context(tc.tile_pool(name="acc", bufs=2))
            sm_p = ctx.enter_context(tc.tile_pool(name="sm", bufs=2))
            yst_p = ctx.enter_context(tc.tile_pool(name="yst", bufs=3))
            ps_qkv = ctx.enter_context(tc.tile_pool(name="ps_qkv", bufs=2, space="PSUM"))
            ps_pv = ctx.enter_context(tc.tile_pool(name="ps_pv", bufs=1, space="PSUM"))
            ps_st = ctx.enter_context(tc.tile_pool(name="ps_st", bufs=3, space="PSUM"))
            ps_t = ctx.enter_context(tc.tile_pool(name="ps_t", bufs=1, space="PSUM"))
            ps_sm = ctx.enter_context(tc.tile_pool(name="ps_sm", bufs=1, space="PSUM"))

            # ---- PE warmup: zero matmuls (no DMA deps) ramp the HAM
            # clock while the first input DMAs are in flight. ----
            warm = res.tile([128, 128], bf16, tag="warm")
            nc.vector.memset(warm, 0.0)
            wps = ps_st.tile([128, span], f32, tag="st")
            for _ in range(WARMUP):
                nc.tensor.matmul(wps[:, :128], warm, warm, start=True, stop=True)

            # ---- startup DMAs split over 4 queues (issuing engine = queue):
            #   sync/vector: x (span 0 in kt chunks, then span halves)
            #   scalar     : qkv weights    gpsimd: bias, consts, wproj ----
            wstrip = {}

            def w_dma(h, p):
                wt = w_p.tile([128, D], bf16, tag=f"w{h}_{p}", name=f"w{h}_{p}")
                nc.scalar.dma_start(
                    wt, wq_d[(p * nh + h) * 128:(p * nh + h + 1) * 128, :])
                wstrip[(h, p)] = wt

            # x rides the two HW-DGE queues (sync + scalar); the slow
            # SWDGE gpsimd queue only carries small/late tensors.
            xall = res.tile([128, NS * SPW], bf16, tag="xall")
            bq = res.tile([128, 3 * nh], f32, tag="bq")
            tribin = res.tile([128, 128], bf16, tag="tribin")
            identb = res.tile([128, 128], bf16, tag="identb")
            ones16 = res.tile([128, 128], f16, tag="ones16")
            nc.gpsimd.dma_start(bq, bq_d)
            nc.gpsimd.dma_start(tribin, tb_d)
            nc.gpsimd.dma_start(identb, id_d)
            nc.gpsimd.dma_start(ones16, on_d)
            nchunk = min(4, KT)
            ktper = KT // nchunk
            for c in range(nchunk):
                nc.sync.dma_start(
                    xall[:, c * ktper * span:(c + 1) * ktper * span],
                    x_d[:, c * ktper * span:(c + 1) * ktper * span])
            w_dma(0, 0)
            w_dma(0, 1)
            w_dma(0, 2)
            for sp in range(1, NS):
                half = SPW // 2
                nc.sync.dma_start(
                    xall[:, sp * SPW:sp * SPW + half],
                    x_d[:, sp * SPW:sp * SPW + half])
                nc.scalar.dma_start(
                    xall[:, sp * SPW + half:(sp + 1) * SPW],
                    x_d[:, sp * SPW + half:(sp + 1) * SPW])
            for h in range(1, nh):
                for p in range(3):
                    w_dma(h, p)
            wp = []
            for g in range(nh):
                w = wp_p.tile([128, D], bf16, tag=f"wpt{g}")
                nc.gpsimd.dma_start(w, wp_d[g * 128:(g + 1) * 128, :])
                wp.append(w)

            # ---- per-head OT accumulation ----
            OT = [res.tile([128, S], bf16, tag=f"ot{h}", name=f"ot{h}")
                  for h in range(nh)]

            # ---- deferred-PE work queue: QKV matmuls of the next head
            # (and early proj stripes during the last head) are emitted
            # one per attention block to fill exp-gated PE idle time. ----
            work_q = deque()

            def pull(n):
                for _ in range(min(n, len(work_q))):
                    work_q.popleft()()

            def drain_q():
                while work_q:
                    work_q.popleft()()

            def enqueue_qkv(h):
                wts = [wstrip.pop((h, p)) for p in range(3)]
                qt = qk_p.tile([128, S], bf16, tag="qt", name=f"qt{h}")
                kt_ = qk_p.tile([128, S], bf16, tag="kt_", name=f"kt_{h}")
                vt = qk_p.tile([128, S], bf16, tag="vt", name=f"vt{h}")
                qkvT = [qt, kt_, vt]
                for sp in range(NS):
                    for p in range(3):
                        cell = {}
                        for kt in range(KT):
                            def mm(kt=kt, sp=sp, wt=wts[p], cell=cell):
                                if kt == 0:
                                    cell['ps'] = ps_qkv.tile(
                                        [128, span], f32, tag="qkv",
                                        name=f"qg{sp}")
                                nc.tensor.matmul(
                                    cell['ps'], wt[:, kt * 128:(kt + 1) * 128],
                                    xall[:, sp * SPW + kt * span:
                                         sp * SPW + (kt + 1) * span],
                                    start=(kt == 0), stop=(kt == KT - 1))
                            work_q.append(mm)

                        def ev(sp=sp, hp=p * nh + h, cell=cell, dst=qkvT[p]):
                            nc.vector.tensor_scalar(
                                out=dst[:, sp * span:(sp + 1) * span],
                                in0=cell['ps'], scalar1=bq[:, hp:hp + 1],
                                scalar2=None, op0=Alu.add)
                        work_q.append(ev)
                return qkvT

            # proj: one stripe = 4 head-matmuls + cast + dma for (dc, sp)
            proj_done = set()

            def proj_stripe(dc, sp, thunks, pool=None, ptag="qkv"):
                cell = {}
                if pool is None:
                    pool = ps_qkv
                for g in range(nh):
                    def mm(g=g, dc=dc, sp=sp, cell=cell, pool=pool, ptag=ptag):
                        if g == 0:
                            cell['ps'] = pool.tile([128, span], f32,
                                                   tag=ptag, name=f"pj{dc}_{sp}")
                        nc.tensor.matmul(
                            cell['ps'], wp[g][:, dc * 128:(dc + 1) * 128],
                            OT[g][:, sp * span:(sp + 1) * span],
                            start=(g == 0), stop=(g == nh - 1))
                    thunks.append(mm)

                def ev(dc=dc, sp=sp, cell=cell):
                    yst = yst_p.tile([128, span], f16, tag="yst",
                                     name=f"yst{dc}_{sp}")
                    nc.vector.tensor_copy(yst, cell['ps'])
                    nc.sync.dma_start(
                        yt_d[dc * 128:(dc + 1) * 128,
                             sp * span:(sp + 1) * span], yst)
                thunks.append(ev)
                proj_done.add((dc, sp))

            # ---- head loop ----
            qkvT_next = enqueue_qkv(0)
            drain_q()
            for h in range(nh):
                QT, KTt, VT = qkvT_next
                if h + 1 < nh:
                    qkvT_next = enqueue_qkv(h + 1)

                vh = v_p.tile([128, S], bf16, tag="v")

                def transpose_batch(tg0, tgn):
                    ps = ps_t.tile([128, 1024], bf16, tag="tp")
                    for j in range(tgn - tg0):
                        nc.tensor.transpose(
                            ps[:, j * 128:(j + 1) * 128],
                            VT[:, (tg0 + j) * 128:(tg0 + j + 1) * 128], identb)
                    nc.vector.tensor_copy(
                        vh[:, tg0 * 128:tgn * 128], ps[:, :(tgn - tg0) * 128])

                transpose_batch(0, min(8, TT))

                for sp in range(NS):
                    if sp == 2 and TT > 8:
                        transpose_batch(8, TT)
                    nkj = KPS * (sp + 1)   # causal: k-blocks 0..nkj-1
                    ps_o = ps_pv.tile([128, span], f32, tag="pv")
                    acc = acc_p.tile([128, span], f16, tag="acc")
                    pend = []  # (kj, pt, qoff) awaiting PV emission

                    def flush_one(nkj=None, ps_o=None):
                        kj, pt, qoff = pend.pop(0)
                        nc.tensor.matmul(
                            ps_o[:, qoff:], vh[:, kj * 128:(kj + 1) * 128],
                            pt[:, qoff:], start=(kj == 0), stop=(kj == nkj - 1))

                    for kj in range(nkj):
                        qoff = max(0, (kj - KPS * sp)) * 128
                        st = ps_st.tile([128, span], f32, tag="st")
                        nc.tensor.matmul(
                            st[:, qoff:], KTt[:, kj * 128:(kj + 1) * 128],
                            QT[:, sp * span + qoff:(sp + 1) * span],
                            start=True, stop=True)
                        pull(1)
                        pt = pt_p.tile([128, span], bf16, tag="pt")
                        nc.scalar.activation(
                            pt[:, qoff:], st[:, qoff:], Act.Exp, scale=scale)
                        if kj >= KPS * sp:  # diagonal block: causal mask
                            nc.gpsimd.tensor_tensor(
                                out=pt[:, qoff:qoff + 128],
                                in0=pt[:, qoff:qoff + 128],
                                in1=tribin, op=Alu.mult)
                        if kj == 0:
                            nc.vector.tensor_copy(acc, pt)
                        else:
                            nc.vector.tensor_tensor(
                                out=acc[:, qoff:], in0=acc[:, qoff:],
                                in1=pt[:, qoff:], op=Alu.add)
                        pend.append((kj, pt, qoff))
                        if len(pend) > 4:
                            flush_one(nkj=nkj, ps_o=ps_o)
                    while pend:
                        flush_one(nkj=nkj, ps_o=ps_o)
                    pull(2)

                    ps_s = ps_sm.tile([128, span], f32, tag="sum")
                    nc.tensor.matmul(ps_s, ones16, acc, start=True, stop=True)
                    recipb = sm_p.tile([128, span], f32, tag="recipb")
                    nc.vector.reciprocal_approx_fast(out=recipb, in_=ps_s)
                    nc.vector.tensor_tensor(
                        out=OT[h][:, sp * span:(sp + 1) * span],
                        in0=ps_o, in1=recipb, op=Alu.mult)

                    if h == nh - 1:
                        # inject early proj stripes (bounded by DVE slack)
                        if sp == 0:
                            for dc in range(min(8, D // 128)):
                                proj_stripe(dc, 0, work_q)
                        elif sp == 1:
                            for dc in range(8, min(12, D // 128)):
                                proj_stripe(dc, 0, work_q)

                drain_q()

            # ---- remaining output projection (rotating over the now-free
            # score/pv PSUM banks for a deeper eviction pipeline) ----
            rot = [(ps_qkv, "qkv"), (ps_st, "st"), (ps_pv, "pv"),
                   (ps_qkv, "qkv"), (ps_st, "st"), (ps_sm, "sum")]
            ri = 0
            for dc in range(D // 128):
                for sp in range(NS):
                    if (dc, sp) in proj_done:
                        continue
                    thunks = []
                    pool, ptag = rot[ri % len(rot)]
                    ri += 1
                    proj_stripe(dc, sp, thunks, pool=pool, ptag=ptag)
                    for t in thunks:
                        t()

    nc.finalize()
    return nc


def pack_x(xb, S=S, D=D, span=512):
    """[S, D] -> span-major packed [128, NS*KT*span] (bf16 upstream)."""
    NS, KT = S // span, D // 128
    return np.ascontiguousarray(
        xb.reshape(NS, span, KT, 128).transpose(3, 0, 2, 1).reshape(128, -1))


def _prep_core_inputs(x, W_qkv, b_qkv, W_proj, core, S=S, D=D, nh=HPC):
    import ml_dtypes
    bf16 = ml_dtypes.bfloat16
    ngr = NCORES // B
    b, hg = core // ngr, core % ngr
    KT = D // 128
    Dfull = W_qkv.shape[0]

    wq = np.empty((3 * nh * 128, D), dtype=bf16)
    bq = np.zeros((128, 3 * nh), dtype=np.float32)
    for p in range(3):
        for h in range(nh):
            g = hg * nh + h
            col = p * Dfull + g * 128
            blk = W_qkv[:, col:col + 128]            # [D, 128]
            hp = p * nh + h
            wq[hp * 128:(hp + 1) * 128] = (
                blk.reshape(KT, 128, 128).transpose(1, 0, 2).reshape(128, D)
                .astype(bf16))
            bq[:, hp] = b_qkv[col:col + 128]
    wp = W_proj[hg * nh * 128:(hg + 1) * nh * 128, :].astype(bf16)

    r = np.arange(128)
    tribin = np.where(r[:, None] <= r[None, :], 1.0, 0.0)
    return {
        "xt": pack_x(x[b]).astype(bf16),
        "wqkv": wq,
        "bqkv": bq,
        "wproj": wp,
        "tribin": tribin.astype(bf16),
        "identb": np.eye(128, dtype=bf16),
        "ones16": np.ones((128, 128), dtype=np.float16),
    }


_CACHE = {}


def kernel(x, W_qkv, b_qkv, W_proj, b_proj, mask):
    from concourse.bass_utils import run_bass_kernel_spmd

    x = np.asarray(x)
    W_qkv = np.asarray(W_qkv)
    b_qkv = np.asarray(b_qkv)
    W_proj = np.asarray(W_proj)
    b_proj = np.asarray(b_proj)

    if "nc" not in _CACHE:
        _CACHE["nc"] = build_nc()
    nc = _CACHE["nc"]

    in_maps = [_prep_core_inputs(x, W_qkv, b_qkv, W_proj, c)
               for c in range(NCORES)]
    res = run_bass_kernel_spmd(nc, in_maps, core_ids=list(range(NCORES)))

    ngr = NCORES // B
    out = np.empty((B, S, D), dtype=np.float32)
    for b in range(B):
        acc = res.results[b * ngr]["yt"].astype(np.float32)
        for g in range(1, ngr):
            acc = acc + res.results[b * ngr + g]["yt"]
        out[b] = acc.T + b_proj[None, :]
    return out
